# revision 26
# baseline (speedup 1.0000x reference)
"""Single-head attention (no 1/sqrt(d) scaling) for Trainium2, 8 NeuronCores.

Problem: x [8, 2048, 768], W [2304, 768], b [2304]
    qkv = x @ W.T + b ; q,k,v = split(qkv)
    out = softmax(q @ k.T) @ v            -> [8, 2048, 768] fp32

Sharding: data-parallel over batch, one batch element per core.

Weight folding (host-side, exact): softmax over keys m is invariant to
per-query constants, so with gm = Wq.T @ Wk and a = Wk.T @ bq,
    S'[n,m] = (x @ gm + a) @ x.T
satisfies softmax(S') == softmax(q @ k.T) row-for-row. This deletes the
entire k projection (1/3 of the QKV GEMM work) from the device.

All matmuls run in fp32r (full PE rate at 512-wide moving operands).

DMA design: every input is ONE large contiguous [128, K] HWDGE
transfer, pre-arranged on the host, ALL on the single SYNC queue in
exact consumption order:
    xs0 | gmc0..gmc5 (column-chunk-major gm) | xs2 | xs1 | wv | xs3
One queue gets the full HBM rate (~400 GB/s after ramp) and FIFO
delivery, so each phase-A stage's data lands just before the PE needs
it. (Splitting across the two HWDGE rings starves whichever ring has
less queued — measured 60-90 GB/s on the loser — so everything big
stays on sync.) bcol/bvrep ride the idle gpsimd SWDGE queue. Phase-A
stage order matches the landing order:
    warmup | z0 (gm chunks trickle in) | z2 | z1 | v0 | v2 | v1 | z3 | v3
30 warmup matmuls on a memset tile cover the framework preamble + DMA
ramp (~6.5us preamble; first-dependency lands 14-17.5us depending on
ramp jitter) and keep the HAM clock at 2.4 GHz — one fill matmul per z0
group absorbs gm-chunk chase lag. Any PE idle gap here re-throttles the
PE clock to 1.2 GHz for ~3.4us (HAM MID window), so warmup errs long.

Phase B per 512-wide n-slice (unchanged):
    S'^T[m,n] = x z (lhsT = resident x^T blocks, 6 accumulating MMs)
    P = exp(S'^T)             (ACT)
    U^T += v_m^T @ P          (6 PSUM banks over 16 m-chunks)
    acc += P                  (DVE denominator partials)
    r = partition_all_reduce(acc); out^T slice = U^T * approx(1/r)
Last slice skips normalization: U banks are copied out as bf16 and
stored with raw acc; the host divides during the gather.
"""

import contextlib

import numpy as np

import concourse.bacc as bacc
import concourse.bass_isa as bass_isa
import concourse.mybir as mybir
import concourse.tile as tile
from concourse.bass_utils import run_bass_kernel_spmd

F32 = mybir.dt.float32
F32R = mybir.dt.float32r
BF16 = mybir.dt.bfloat16
AF = mybir.ActivationFunctionType
ALU = mybir.AluOpType

B, N, H = 8, 2048, 768
P = 128
ND = H // P      # 6 h-chunks
NM = N // P      # 16 m-chunks
SL = 512         # n-slice width (fp32 moving-operand max / one PSUM bank)
NSL = N // SL    # 4 n-slices
TAIL_SLICE = NSL - 1


def build_nc(loop_iters=None, split=1, nm_eff=NM,
             fast_recip=True, host_tail=True, warmup=27,
             fill_z0=1, post_fill=0, tail_bf16=True, act_pad=0):
    nc = bacc.Bacc("TRN2", target_bir_lowering=False, debug=False)

    HSL = SL // 2
    xs_d = [nc.dram_tensor(f"xs{s}", [P, ND * SL], F32R, kind="ExternalInput")
            for s in range(NSL)]
    gm_d = [nc.dram_tensor(f"gmc{hc}", [P, H], F32R, kind="ExternalInput")
            for hc in range(ND)]
    wv_d = nc.dram_tensor("wv", [P, ND * H], F32R, kind="ExternalInput")
    bcol = nc.dram_tensor("bcol", [P, ND], F32, kind="ExternalInput")
    bvrep = nc.dram_tensor("bvrep", [P, H], F32, kind="ExternalInput")
    out = nc.dram_tensor("out", [H, N], F32, kind="ExternalOutput")  # transposed
    if host_tail:
        udt = BF16 if tail_bf16 else F32
        ulast = nc.dram_tensor("ulast", [H, SL], udt, kind="ExternalOutput")
        racc = nc.dram_tensor("racc", [P, SL], F32, kind="ExternalOutput")

    def mm_group(psum, lhs_list, rhs_slicer, split=1):
        width = psum.shape[-1]
        hw = width // split
        n = len(lhs_list)
        steps = [(c, h) for c in range(n) for h in range(split)]
        for idx, (c, h) in enumerate(steps):
            lo = h * hw
            nc.tensor.matmul(
                psum[:, lo : lo + hw], lhs_list[c], rhs_slicer(c, lo, hw),
                start=(idx == 0), stop=(idx == len(steps) - 1),
            )

    with tile.TileContext(nc) as tc:
        with (
            tc.tile_pool(name="dram", bufs=1, space="DRAM") as dram,
            tc.tile_pool(name="const", bufs=1) as const,
            tc.tile_pool(name="keep", bufs=1) as keep,
            tc.For_i(0, loop_iters, 1) if loop_iters else contextlib.nullcontext(),
        ):
            bcol_sb = const.tile([P, ND], F32)

            # resident: all of x^T (4 slabs) + v (16 tiles) + z strips (24)
            xsl = [keep.tile([P, ND * SL], F32R, name=f"xs{s}") for s in range(NSL)]

            def xr(c, s):
                return xsl[s][:, c * SL : (c + 1) * SL]

            def xblk(c, s, blk):
                lo = c * SL + blk * P
                return xsl[s][:, lo : lo + P]

            vsb = [keep.tile([P, H], F32R, name=f"v{ni}") for ni in range(NM)]
            zall = [
                [keep.tile([P, SL], F32R, name=f"z{hc}_{s}") for hc in range(ND)]
                for s in range(NSL)
            ]

            with tc.tile_pool(name="xw_pool", bufs=1) as xw:
                gmsb = [xw.tile([P, H], F32R, name=f"gm{hc}") for hc in range(ND)]

                def gslice(c, hc):
                    return gmsb[hc][:, c * P : (c + 1) * P]

                xwa = tc.alloc_tile_pool(name="xwa_pool", bufs=1)
                wvsb = xwa.tile([P, ND * H], F32R, name="wvsb")

                def wv(c):
                    return wvsb[:, c * H : (c + 1) * H]

                bvb = xwa.tile([P, H], F32, name="bvb")
                warm_f32 = xwa.tile([P, SL], F32, name="warm_sb")
                warm_sb = warm_f32[:].bitcast(F32R)

                # ---- startup DMA schedule -----------------------------------
                # Sync (HWDGE) carries every big input as one contiguous
                # transfer, in exact consumption order; it runs alone so it
                # gets the full HBM rate and FIFO landing order. The tiny
                # bias tensors ride the idle gpsimd SWDGE queue.
                nc.gpsimd.memset(warm_f32[:], 0.0)
                nc.gpsimd.dma_start(bcol_sb[:], bcol.ap())
                nc.gpsimd.dma_start(bvb[:], bvrep.ap())

                # ALL big inputs on the single sync HWDGE queue in exact
                # consumption order — one queue gets the full HBM rate and
                # FIFO landing order (a second concurrent queue starves the
                # first and delays the critical z0 prefix).
                nc.sync.dma_start(xsl[0][:], xs_d[0].ap())
                for hc in range(ND):
                    nc.sync.dma_start(gmsb[hc][:], gm_d[hc].ap())
                nc.sync.dma_start(xsl[2][:], xs_d[2].ap())
                nc.sync.dma_start(xsl[1][:], xs_d[1].ap())
                nc.sync.dma_start(wvsb[:], wv_d.ap())
                nc.sync.dma_start(xsl[3][:], xs_d[3].ap())

                # ---- Phase A: z strips + v projection -----------------------
                # Stage order matches the DMA landing order above; warm-fill
                # matmuls cover the preamble and the z0->z2 DMA bubble so the
                # HAM clock never drops.
                with (
                    tc.tile_pool(name="vps", bufs=2, space="PSUM") as vps,
                    tc.tile_pool(name="zps", bufs=2, space="PSUM") as zps,
                    tc.tile_pool(name="wmps", bufs=1, space="PSUM") as wmps,
                ):
                    wps = wmps.tile([P, SL], F32, name="warm_ps")

                    def warm(k):
                        for _ in range(k):
                            nc.tensor.matmul(
                                wps[:], warm_sb[:, 0:P], warm_sb,
                                start=True, stop=True,
                            )

                    warm(warmup)

                    def z_proj(zs, fill=0):
                        for hc in range(ND):
                            ps = zps.tile([P, SL], F32, name="z_ps", tag="z")
                            mm_group(
                                ps, [gslice(c, hc) for c in range(ND)],
                                lambda c, lo, w, _s=zs: xr(c, _s)[:, lo : lo + w],
                                split=split,
                            )
                            nc.scalar.activation(
                                zall[zs][hc][:], ps[:], AF.Identity,
                                bias=bcol_sb[:, hc : hc + 1],
                            )
                            warm(fill)

                    def v_proj(s):
                        for blk in range(NSL):
                            ni = s * NSL + blk
                            pa = vps.tile([P, SL], F32, name="pa", tag="pa")
                            pb = vps.tile([P, H - SL], F32, name="pb", tag="pb")
                            mm_group(
                                pa, [xblk(c, s, blk) for c in range(ND)],
                                lambda c, lo, w: wv(c)[:, lo : lo + w],
                                split=split,
                            )
                            mm_group(
                                pb, [xblk(c, s, blk) for c in range(ND)],
                                lambda c, lo, w: wv(c)[:, SL + lo : SL + lo + w],
                            )
                            nc.vector.tensor_tensor(
                                vsb[ni][:, 0:SL], pa[:], bvb[:, 0:SL], op=ALU.add
                            )
                            nc.vector.tensor_tensor(
                                vsb[ni][:, SL:H], pb[:], bvb[:, SL:H], op=ALU.add
                            )

                    z_proj(0, fill=fill_z0)
                    warm(post_fill)
                    z_proj(2)
                    z_proj(1)
                    v_proj(0)
                    v_proj(2)
                    v_proj(1)
                    z_proj(3)
                    v_proj(3)

                xwa.release()

                # ---- Phase B: attention (software-pipelined m-loop) ----
                with (
                    tc.tile_pool(name="p_pool", bufs=5) as p_pool,
                    tc.tile_pool(name="u_ps", bufs=1, space="PSUM") as u_ps,
                    tc.tile_pool(name="sps", bufs=2, space="PSUM") as sps,
                    tc.tile_pool(name="usb_pool", bufs=1) as usb_pool,
                    tc.tile_pool(name="misc", bufs=1) as misc,
                ):
                    # Scalar's exit code sits past the 256-instruction IRAM
                    # block boundary; pad ACT with no-op copies mid-kernel so
                    # the second IRAM block is fetched while the fetch is
                    # hidden, not at the exit branch (~5us stall otherwise).
                    pad_sb = None
                    if act_pad:
                        pad_sb = misc.tile([P, 2], F32, name="pad_sb", tag="pad")
                        nc.vector.memset(pad_sb[:], 0.0)
                    pads_left = act_pad

                    for ns in range(NSL):
                        tail = host_tail and ns == TAIL_SLICE
                        zbuf = zall[ns]
                        us = [
                            u_ps.tile([P, SL], F32, name=f"u{c}", tag=f"u{c}")
                            for c in range(ND)
                        ]
                        acc = misc.tile([P, SL], F32, name="acc", tag="acc", bufs=2)

                        p_sbs = [None] * NM
                        for mi in range(nm_eff + 1):
                            if mi < nm_eff:
                                s, blk = divmod(mi, NSL)
                                s_ps = sps.tile([P, SL], F32, name="s_ps", tag="s")
                                mm_group(
                                    s_ps, [xblk(c, s, blk) for c in range(ND)],
                                    lambda c, lo, w: zbuf[c][:, lo : lo + w],
                                    split=split,
                                )
                                p_sb = p_pool.tile([P, SL], F32R, name="p_sb", tag="p")
                                nc.scalar.activation(p_sb[:], s_ps[:], AF.Exp)
                                p_sbs[mi] = p_sb
                                if 1 <= ns <= 2 and pads_left > 0:
                                    for _ in range(2):
                                        if pads_left > 0:
                                            nc.scalar.copy(
                                                pad_sb[:, 0:1], pad_sb[:, 1:2]
                                            )
                                            pads_left -= 1
                            if mi >= 1:
                                j = mi - 1
                                pj = p_sbs[j]
                                if j == 0:
                                    nc.vector.tensor_copy(acc[:], pj[:])
                                else:
                                    nc.vector.tensor_tensor(
                                        acc[:], pj[:], acc[:], op=ALU.add
                                    )
                                for c in range(ND):
                                    nc.tensor.matmul(
                                        us[c][:],
                                        vsb[j][:, c * P : (c + 1) * P],
                                        pj[:],
                                        start=(j == 0),
                                        stop=(j == nm_eff - 1),
                                    )
                                p_sbs[j] = None

                        # copy-then-scale eviction: raw copies (DVE, plus ACT
                        # on the tail) free the U PSUM banks right after the
                        # last matmul, so the next slice's U accumulation
                        # never waits on the allreduce -> reciprocal ->
                        # multiply chain
                        if tail:
                            nc.sync.dma_start(racc.ap(), acc[:])
                        u_sbs = []
                        udt = BF16 if (tail and tail_bf16) else F32
                        for c in range(ND):
                            u_sb = usb_pool.tile(
                                [P, SL], udt, name=f"usb{c}", tag=f"usb{c}"
                            )
                            if tail and c % 2 == 1:
                                # no later exp to delay — ACT halves the tail
                                nc.scalar.copy(u_sb[:], us[c][:])
                            else:
                                nc.vector.tensor_copy(u_sb[:], us[c][:])
                            u_sbs.append(u_sb)
                        if tail:
                            # unnormalized exit; host divides at the gather
                            for c in range(ND):
                                store_eng = nc.sync if c % 2 == 0 else nc.scalar
                                store_eng.dma_start(
                                    ulast.ap()[c * P : (c + 1) * P, :], u_sbs[c][:]
                                )
                        else:
                            rall = misc.tile([P, SL], F32, name="rall", tag="rall")
                            nc.gpsimd.partition_all_reduce(
                                rall[:], acc[:], P, bass_isa.ReduceOp.add
                            )
                            rinv = misc.tile([P, SL], F32, name="rinv", tag="rinv")
                            if fast_recip:
                                nc.vector.reciprocal_approx_fast(rinv[:], rall[:])
                            else:
                                nc.vector.reciprocal(rinv[:], rall[:])
                            for c in range(ND):
                                nc.vector.tensor_tensor(
                                    u_sbs[c][:], u_sbs[c][:], rinv[:], op=ALU.mult
                                )
                                nc.sync.dma_start(
                                    out.ap()[c * P : (c + 1) * P, ns * SL : (ns + 1) * SL],
                                    u_sbs[c][:],
                                )

    nc.compile()
    return nc


_NC = None


def make_in_maps(x, W, b):
    x = np.ascontiguousarray(x, dtype=np.float32)
    W = np.asarray(W, dtype=np.float32)
    b = np.asarray(b, dtype=np.float32)
    Wq, Wk, Wv = W[:H], W[H : 2 * H], W[2 * H :]
    bq = b[:H]
    gm_host = np.ascontiguousarray(Wq.T @ Wk)                  # [768, 768]
    a = Wk.T @ bq                                              # [768]
    # gm column-chunk-major: gmc[hc][p, c*128+col] = gm[c*128+p, hc*128+col]
    gm3 = gm_host.reshape(ND, P, H)                            # [c, p, col]
    gmcs = {
        f"gmc{hc}": np.ascontiguousarray(
            gm3[:, :, hc * P : (hc + 1) * P].transpose(1, 0, 2).reshape(P, H)
        )
        for hc in range(ND)
    }
    wvT3 = np.ascontiguousarray(Wv.T).reshape(ND, P, H)        # [c, p, j]
    wv_dev = np.ascontiguousarray(wvT3.transpose(1, 0, 2).reshape(P, ND * H))
    bcol = np.ascontiguousarray(a.reshape(ND, P).T)            # [128, 6]
    bvrep = np.ascontiguousarray(
        np.broadcast_to(b[2 * H :].reshape(1, H), (P, H))
    )
    maps = []
    for i in range(B):
        xT3 = np.ascontiguousarray(x[i].T).reshape(ND, P, N)   # [c, p, n]
        m = {
            "wv": wv_dev,
            "bcol": bcol,
            "bvrep": bvrep,
        }
        m.update(gmcs)
        for s in range(NSL):
            m[f"xs{s}"] = np.ascontiguousarray(
                xT3[:, :, s * SL : (s + 1) * SL].transpose(1, 0, 2).reshape(P, ND * SL)
            )
        maps.append(m)
    return maps


def kernel(x: np.ndarray, W: np.ndarray, b: np.ndarray) -> np.ndarray:
    global _NC
    if _NC is None:
        _NC = build_nc()

    in_maps = make_in_maps(x, W, b)
    res = run_bass_kernel_spmd(_NC, in_maps, core_ids=list(range(B)))
    outs = []
    for i in range(B):
        oT = np.array(res.results[i]["out"])                   # [768, 2048]
        if "ulast" in res.results[i]:
            ul = np.asarray(res.results[i]["ulast"], dtype=np.float64)
            ra = np.asarray(res.results[i]["racc"], dtype=np.float64)
            r = ra.sum(axis=0)                                 # [512]
            lo = TAIL_SLICE * SL
            oT[:, lo : lo + SL] = (ul / r[None, :]).astype(np.float32)
        outs.append(np.ascontiguousarray(oT.T))
    return np.stack(outs, axis=0)


# revision 29
# speedup vs baseline: 1.0031x; 1.0031x over previous
"""Single-head attention (no 1/sqrt(d) scaling) for Trainium2, 8 NeuronCores.

Problem: x [8, 2048, 768], W [2304, 768], b [2304]
    qkv = x @ W.T + b ; q,k,v = split(qkv)
    out = softmax(q @ k.T) @ v            -> [8, 2048, 768] fp32

Sharding: data-parallel over batch, one batch element per core.

Weight folding (host-side, exact): softmax over keys m is invariant to
per-query constants, so with gm = Wq.T @ Wk and a = Wk.T @ bq,
    S'[n,m] = (x @ gm + a) @ x.T
satisfies softmax(S') == softmax(q @ k.T) row-for-row. This deletes the
entire k projection (1/3 of the QKV GEMM work) from the device.

All matmuls run in fp32r (full PE rate at 512-wide moving operands).

DMA design: every input is ONE large contiguous [128, K] HWDGE
transfer, pre-arranged on the host, ALL on the single SYNC queue in
exact consumption order:
    xs0 | gmc0..gmc5 (column-chunk-major gm) | xs2 | xs1 | wv | xs3
One queue gets the full HBM rate (~400 GB/s after ramp) and FIFO
delivery, so each phase-A stage's data lands just before the PE needs
it. (Splitting across the two HWDGE rings starves whichever ring has
less queued — measured 60-90 GB/s on the loser — so everything big
stays on sync.) bcol/bvrep ride the idle gpsimd SWDGE queue. Phase-A
stage order matches the landing order:
    warmup | z0 (gm chunks trickle in) | z2 | z1 | v0 | v2 | v1 | z3 | v3
30 warmup matmuls on a memset tile cover the framework preamble + DMA
ramp (~6.5us preamble; first-dependency lands 14-17.5us depending on
ramp jitter) and keep the HAM clock at 2.4 GHz — one fill matmul per z0
group absorbs gm-chunk chase lag. Any PE idle gap here re-throttles the
PE clock to 1.2 GHz for ~3.4us (HAM MID window), so warmup errs long.

Phase B per 512-wide n-slice (unchanged):
    S'^T[m,n] = x z (lhsT = resident x^T blocks, 6 accumulating MMs)
    P = exp(S'^T)             (ACT)
    U^T += v_m^T @ P          (6 PSUM banks over 16 m-chunks)
    acc += P                  (DVE denominator partials)
    r = partition_all_reduce(acc); out^T slice = U^T * approx(1/r)
Last slice skips normalization: U banks are copied out as bf16 and
stored with raw acc; the host divides during the gather.
"""

import contextlib

import numpy as np

import concourse.bacc as bacc
import concourse.bass_isa as bass_isa
import concourse.mybir as mybir
import concourse.tile as tile
from concourse.bass_utils import run_bass_kernel_spmd

F32 = mybir.dt.float32
F32R = mybir.dt.float32r
BF16 = mybir.dt.bfloat16
AF = mybir.ActivationFunctionType
ALU = mybir.AluOpType

B, N, H = 8, 2048, 768
P = 128
ND = H // P      # 6 h-chunks
NM = N // P      # 16 m-chunks
SL = 512         # n-slice width (fp32 moving-operand max / one PSUM bank)
NSL = N // SL    # 4 n-slices
TAIL_SLICE = NSL - 1


def build_nc(loop_iters=None, split=1, nm_eff=NM,
             fast_recip=True, host_tail=True, warmup=30,
             fill_z0=1, post_fill=0, tail_bf16=True, act_pad=0):
    nc = bacc.Bacc("TRN2", target_bir_lowering=False, debug=False)

    HSL = SL // 2
    xs_d = [nc.dram_tensor(f"xs{s}", [P, ND * SL], F32R, kind="ExternalInput")
            for s in range(NSL)]
    gm_d = [nc.dram_tensor(f"gmc{hc}", [P, H], F32R, kind="ExternalInput")
            for hc in range(ND)]
    wv_d = nc.dram_tensor("wv", [P, ND * H], F32R, kind="ExternalInput")
    bcol = nc.dram_tensor("bcol", [P, ND], F32, kind="ExternalInput")
    bvrep = nc.dram_tensor("bvrep", [P, H], F32, kind="ExternalInput")
    out = nc.dram_tensor("out", [H, N], F32, kind="ExternalOutput")  # transposed
    if host_tail:
        udt = BF16 if tail_bf16 else F32
        ulast = nc.dram_tensor("ulast", [H, SL], udt, kind="ExternalOutput")
        racc = nc.dram_tensor("racc", [P, SL], F32, kind="ExternalOutput")

    def mm_group(psum, lhs_list, rhs_slicer, split=1):
        width = psum.shape[-1]
        hw = width // split
        n = len(lhs_list)
        steps = [(c, h) for c in range(n) for h in range(split)]
        for idx, (c, h) in enumerate(steps):
            lo = h * hw
            nc.tensor.matmul(
                psum[:, lo : lo + hw], lhs_list[c], rhs_slicer(c, lo, hw),
                start=(idx == 0), stop=(idx == len(steps) - 1),
            )

    with tile.TileContext(nc) as tc:
        with (
            tc.tile_pool(name="dram", bufs=1, space="DRAM") as dram,
            tc.tile_pool(name="const", bufs=1) as const,
            tc.tile_pool(name="keep", bufs=1) as keep,
            tc.For_i(0, loop_iters, 1) if loop_iters else contextlib.nullcontext(),
        ):
            bcol_sb = const.tile([P, ND], F32)

            # resident: all of x^T (4 slabs) + v (16 tiles) + z strips (24)
            xsl = [keep.tile([P, ND * SL], F32R, name=f"xs{s}") for s in range(NSL)]

            def xr(c, s):
                return xsl[s][:, c * SL : (c + 1) * SL]

            def xblk(c, s, blk):
                lo = c * SL + blk * P
                return xsl[s][:, lo : lo + P]

            vsb = [keep.tile([P, H], F32R, name=f"v{ni}") for ni in range(NM)]
            zall = [
                [keep.tile([P, SL], F32R, name=f"z{hc}_{s}") for hc in range(ND)]
                for s in range(NSL)
            ]

            with tc.tile_pool(name="xw_pool", bufs=1) as xw:
                gmsb = [xw.tile([P, H], F32R, name=f"gm{hc}") for hc in range(ND)]

                def gslice(c, hc):
                    return gmsb[hc][:, c * P : (c + 1) * P]

                # phase-B pools allocated up-front (pool releases are LIFO;
                # these outlive the phase-A pools). sps doubles as the
                # warmup-matmul PSUM target, so phase A fits in 8 banks:
                # vps 4 + zps 2 + sps 2.
                sps = tc.alloc_tile_pool(name="sps", bufs=2, space="PSUM")
                p_pool = tc.alloc_tile_pool(name="p_pool", bufs=5)

                xwa = tc.alloc_tile_pool(name="xwa_pool", bufs=1)
                wvsb = xwa.tile([P, ND * H], F32R, name="wvsb")

                def wv(c):
                    return wvsb[:, c * H : (c + 1) * H]

                bvb = xwa.tile([P, H], F32, name="bvb")
                warm_f32 = xwa.tile([P, SL], F32, name="warm_sb")
                warm_sb = warm_f32[:].bitcast(F32R)

                # ---- startup DMA schedule -----------------------------------
                # Sync (HWDGE) carries every big input as one contiguous
                # transfer, in exact consumption order; it runs alone so it
                # gets the full HBM rate and FIFO landing order. The tiny
                # bias tensors ride the idle gpsimd SWDGE queue.
                nc.gpsimd.memset(warm_f32[:], 0.0)
                nc.gpsimd.dma_start(bcol_sb[:], bcol.ap())
                nc.gpsimd.dma_start(bvb[:], bvrep.ap())

                # ALL big inputs on the single sync HWDGE queue in exact
                # consumption order — one queue gets the full HBM rate and
                # FIFO landing order (a second concurrent queue starves the
                # first and delays the critical z0 prefix).
                nc.sync.dma_start(xsl[0][:], xs_d[0].ap())
                for hc in range(ND):
                    nc.sync.dma_start(gmsb[hc][:], gm_d[hc].ap())
                nc.sync.dma_start(xsl[2][:], xs_d[2].ap())
                nc.sync.dma_start(xsl[1][:], xs_d[1].ap())
                nc.sync.dma_start(wvsb[:], wv_d.ap())
                nc.sync.dma_start(xsl[3][:], xs_d[3].ap())

                # ---- Phase A: z strips + v projection -----------------------
                # Stage order matches the DMA landing order above; warm-fill
                # matmuls cover the preamble and the z0->z2 DMA bubble so the
                # HAM clock never drops.
                with (
                    tc.tile_pool(name="vps", bufs=2, space="PSUM") as vps,
                    tc.tile_pool(name="zps", bufs=2, space="PSUM") as zps,
                ):
                    def warm(k):
                        for _ in range(k):
                            wps = sps.tile([P, SL], F32, name="s_ps", tag="s")
                            nc.tensor.matmul(
                                wps[:], warm_sb[:, 0:P], warm_sb,
                                start=True, stop=True,
                            )

                    warm(warmup)

                    def z_proj(zs, fill=0):
                        for hc in range(ND):
                            ps = zps.tile([P, SL], F32, name="z_ps", tag="z")
                            mm_group(
                                ps, [gslice(c, hc) for c in range(ND)],
                                lambda c, lo, w, _s=zs: xr(c, _s)[:, lo : lo + w],
                                split=split,
                            )
                            nc.scalar.activation(
                                zall[zs][hc][:], ps[:], AF.Identity,
                                bias=bcol_sb[:, hc : hc + 1],
                            )
                            warm(fill)

                    def v_proj(s):
                        for blk in range(NSL):
                            ni = s * NSL + blk
                            pa = vps.tile([P, SL], F32, name="pa", tag="pa")
                            pb = vps.tile([P, H - SL], F32, name="pb", tag="pb")
                            mm_group(
                                pa, [xblk(c, s, blk) for c in range(ND)],
                                lambda c, lo, w: wv(c)[:, lo : lo + w],
                                split=split,
                            )
                            mm_group(
                                pb, [xblk(c, s, blk) for c in range(ND)],
                                lambda c, lo, w: wv(c)[:, SL + lo : SL + lo + w],
                            )
                            nc.vector.tensor_tensor(
                                vsb[ni][:, 0:SL], pa[:], bvb[:, 0:SL], op=ALU.add
                            )
                            nc.vector.tensor_tensor(
                                vsb[ni][:, SL:H], pb[:], bvb[:, SL:H], op=ALU.add
                            )

                    z_proj(0, fill=fill_z0)
                    warm(post_fill)
                    z_proj(2)
                    z_proj(1)
                    v_proj(0)
                    v_proj(2)
                    v_proj(1)
                    z_proj(3)
                    # hoist slice-0's first S' group + exp ahead of v3: its
                    # exp then runs on ACT under v3's 8.5us of matmuls, so
                    # phase B's first U group starts with no S'->exp->U
                    # dependency bubble.
                    s_ps0 = sps.tile([P, SL], F32, name="s_ps", tag="s")
                    mm_group(
                        s_ps0, [xblk(c, 0, 0) for c in range(ND)],
                        lambda c, lo, w: zall[0][c][:, lo : lo + w],
                        split=split,
                    )
                    p0_carry = p_pool.tile([P, SL], F32R, name="p_sb", tag="p")
                    nc.scalar.activation(p0_carry[:], s_ps0[:], AF.Exp)
                    v_proj(3)

                xwa.release()

                # ---- Phase B: attention (software-pipelined m-loop) ----
                with (
                    tc.tile_pool(name="u_ps", bufs=1, space="PSUM") as u_ps,
                    tc.tile_pool(name="usb_pool", bufs=1) as usb_pool,
                    tc.tile_pool(name="misc", bufs=1) as misc,
                ):
                    # Scalar's exit code sits past the 256-instruction IRAM
                    # block boundary; pad ACT with no-op copies mid-kernel so
                    # the second IRAM block is fetched while the fetch is
                    # hidden, not at the exit branch (~5us stall otherwise).
                    pad_sb = None
                    if act_pad:
                        pad_sb = misc.tile([P, 2], F32, name="pad_sb", tag="pad")
                        nc.vector.memset(pad_sb[:], 0.0)
                    pads_left = act_pad

                    for ns in range(NSL):
                        tail = host_tail and ns == TAIL_SLICE
                        zbuf = zall[ns]
                        us = [
                            u_ps.tile([P, SL], F32, name=f"u{c}", tag=f"u{c}")
                            for c in range(ND)
                        ]
                        acc = misc.tile([P, SL], F32, name="acc", tag="acc", bufs=2)

                        p_sbs = [None] * NM
                        for mi in range(nm_eff + 1):
                            if mi < nm_eff:
                                if ns == 0 and mi == 0:
                                    p_sbs[0] = p0_carry
                                    continue
                                s, blk = divmod(mi, NSL)
                                s_ps = sps.tile([P, SL], F32, name="s_ps", tag="s")
                                mm_group(
                                    s_ps, [xblk(c, s, blk) for c in range(ND)],
                                    lambda c, lo, w: zbuf[c][:, lo : lo + w],
                                    split=split,
                                )
                                p_sb = p_pool.tile([P, SL], F32R, name="p_sb", tag="p")
                                nc.scalar.activation(p_sb[:], s_ps[:], AF.Exp)
                                p_sbs[mi] = p_sb
                                if 1 <= ns <= 2 and pads_left > 0:
                                    for _ in range(2):
                                        if pads_left > 0:
                                            nc.scalar.copy(
                                                pad_sb[:, 0:1], pad_sb[:, 1:2]
                                            )
                                            pads_left -= 1
                            if mi >= 1:
                                j = mi - 1
                                pj = p_sbs[j]
                                if j == 0:
                                    nc.vector.tensor_copy(acc[:], pj[:])
                                else:
                                    nc.vector.tensor_tensor(
                                        acc[:], pj[:], acc[:], op=ALU.add
                                    )
                                for c in range(ND):
                                    nc.tensor.matmul(
                                        us[c][:],
                                        vsb[j][:, c * P : (c + 1) * P],
                                        pj[:],
                                        start=(j == 0),
                                        stop=(j == nm_eff - 1),
                                    )
                                p_sbs[j] = None

                        # copy-then-scale eviction: raw copies (DVE, plus ACT
                        # on the tail) free the U PSUM banks right after the
                        # last matmul, so the next slice's U accumulation
                        # never waits on the allreduce -> reciprocal ->
                        # multiply chain
                        if tail:
                            nc.sync.dma_start(racc.ap(), acc[:])
                        u_sbs = []
                        udt = BF16 if (tail and tail_bf16) else F32
                        for c in range(ND):
                            u_sb = usb_pool.tile(
                                [P, SL], udt, name=f"usb{c}", tag=f"usb{c}"
                            )
                            if tail and c % 2 == 1:
                                # no later exp to delay — ACT halves the tail
                                nc.scalar.copy(u_sb[:], us[c][:])
                            else:
                                nc.vector.tensor_copy(u_sb[:], us[c][:])
                            u_sbs.append(u_sb)
                        if tail:
                            # unnormalized exit; host divides at the gather
                            for c in range(ND):
                                store_eng = nc.sync if c % 2 == 0 else nc.scalar
                                store_eng.dma_start(
                                    ulast.ap()[c * P : (c + 1) * P, :], u_sbs[c][:]
                                )
                        else:
                            rall = misc.tile([P, SL], F32, name="rall", tag="rall")
                            nc.gpsimd.partition_all_reduce(
                                rall[:], acc[:], P, bass_isa.ReduceOp.add
                            )
                            rinv = misc.tile([P, SL], F32, name="rinv", tag="rinv")
                            if fast_recip:
                                nc.vector.reciprocal_approx_fast(rinv[:], rall[:])
                            else:
                                nc.vector.reciprocal(rinv[:], rall[:])
                            for c in range(ND):
                                nc.vector.tensor_tensor(
                                    u_sbs[c][:], u_sbs[c][:], rinv[:], op=ALU.mult
                                )
                                nc.sync.dma_start(
                                    out.ap()[c * P : (c + 1) * P, ns * SL : (ns + 1) * SL],
                                    u_sbs[c][:],
                                )

                p_pool.release()
                sps.release()

    nc.compile()
    return nc


_NC = None


def make_in_maps(x, W, b):
    x = np.ascontiguousarray(x, dtype=np.float32)
    W = np.asarray(W, dtype=np.float32)
    b = np.asarray(b, dtype=np.float32)
    Wq, Wk, Wv = W[:H], W[H : 2 * H], W[2 * H :]
    bq = b[:H]
    gm_host = np.ascontiguousarray(Wq.T @ Wk)                  # [768, 768]
    a = Wk.T @ bq                                              # [768]
    # gm column-chunk-major: gmc[hc][p, c*128+col] = gm[c*128+p, hc*128+col]
    gm3 = gm_host.reshape(ND, P, H)                            # [c, p, col]
    gmcs = {
        f"gmc{hc}": np.ascontiguousarray(
            gm3[:, :, hc * P : (hc + 1) * P].transpose(1, 0, 2).reshape(P, H)
        )
        for hc in range(ND)
    }
    wvT3 = np.ascontiguousarray(Wv.T).reshape(ND, P, H)        # [c, p, j]
    wv_dev = np.ascontiguousarray(wvT3.transpose(1, 0, 2).reshape(P, ND * H))
    bcol = np.ascontiguousarray(a.reshape(ND, P).T)            # [128, 6]
    bvrep = np.ascontiguousarray(
        np.broadcast_to(b[2 * H :].reshape(1, H), (P, H))
    )
    maps = []
    for i in range(B):
        xT3 = np.ascontiguousarray(x[i].T).reshape(ND, P, N)   # [c, p, n]
        m = {
            "wv": wv_dev,
            "bcol": bcol,
            "bvrep": bvrep,
        }
        m.update(gmcs)
        for s in range(NSL):
            m[f"xs{s}"] = np.ascontiguousarray(
                xT3[:, :, s * SL : (s + 1) * SL].transpose(1, 0, 2).reshape(P, ND * SL)
            )
        maps.append(m)
    return maps


def kernel(x: np.ndarray, W: np.ndarray, b: np.ndarray) -> np.ndarray:
    global _NC
    if _NC is None:
        _NC = build_nc()

    in_maps = make_in_maps(x, W, b)
    res = run_bass_kernel_spmd(_NC, in_maps, core_ids=list(range(B)))
    outs = []
    for i in range(B):
        oT = np.array(res.results[i]["out"])                   # [768, 2048]
        if "ulast" in res.results[i]:
            ul = np.asarray(res.results[i]["ulast"], dtype=np.float64)
            ra = np.asarray(res.results[i]["racc"], dtype=np.float64)
            r = ra.sum(axis=0)                                 # [512]
            lo = TAIL_SLICE * SL
            oT[:, lo : lo + SL] = (ul / r[None, :]).astype(np.float32)
        outs.append(np.ascontiguousarray(oT.T))
    return np.stack(outs, axis=0)
